# revision 1
# baseline (speedup 1.0000x reference)
"""Trainium2 Bass kernel for GuidedImplicitPointSampler KNN (top-8 + occupancy mask).

Strategy:
  - Shard the N=32768 query points across 8 NeuronCores (4096 each); every core
    holds the full M=16384 target cloud.
  - Per core, compute s[n,m] = 2*q_n.k_m - |k_m|^2 on the PE as a K=4 matmul
    (rows: 2kx,2ky,2kz,-|k|^2 moving; qx,qy,qz,1 stationary).  Since
    d2[n,m] = |q_n|^2 - s[n,m] and |q_n|^2 is constant per row, the 8 nearest
    targets are exactly the 8 LARGEST s values -> hardware top-8 (`nc.vector.max`)
    straight out of PSUM, no full distance matrix ever materialized.
  - K=4 uses only 4 of 128 PE rows, so 4 independent matmuls are packed into
    row-groups 0/32/64/96 via tile_position and run concurrently.
  - Epilogue on the tiny [128, 8] results: d = sqrt(max(q2 - s, 0)), zero rows
    whose nearest distance <= 0.25.
"""

import numpy as np

N = 32768
M = 16384
KNN = 8
OCC_RADIUS = 0.25
N_CORES = 8
NPC = N // N_CORES        # 4096 queries per core
RT = NPC // 128           # 32 row-tiles of 128 queries
CHUNK = 512               # matmul moving free dim (one PSUM bank)
PACK = 4                  # concurrent matmuls in PE row-groups
GROUP = CHUNK * PACK      # 2048 targets per PSUM tile
NGROUP = M // GROUP       # 8 groups per row-tile

_CACHE = {}


def _build(reps=1, mmdt="f16x3"):
    key = ("nc", reps, mmdt)
    if key in _CACHE:
        return _CACHE[key]
    from concourse import bacc, tile, mybir

    dt = mybir.dt
    # mmdt selects the PE path for s = 2q.k - |k|^2 (all run 1 cycle/row
    # except plain f32 which is 4):
    #   "f16k11": fp16 hi/lo split folded into ONE K=11 matmul:
    #            lhsT col = [qh(3), ql(3), qh(3), 1, 1],
    #            rhs col = [kh(3), kh(3), kl(3), -k2h, -k2l]  (error ~2^-22,
    #            same PE rate as K=4 since K<=32 fits one row-group).
    #   "f16x3": same terms as 3 separate accumulating matmuls (slower PE).
    #   "f32r":  fp32 bits at full PE rate, reduced internal precision (~2^-18).
    #   "f32":   exact fp32, 4x slower PE.
    mdt = {"f32r": dt.float32r, "f32": dt.float32, "f16x3": dt.float16,
           "f16k11": dt.float16}[mmdt]
    KDIM = 11 if mmdt == "f16k11" else 4
    nc = bacc.Bacc("TRN2", target_bir_lowering=False, debug=False,
                   num_devices=N_CORES)

    if mmdt == "f16x3":
        lhsA_d = nc.dram_tensor("lhsA", [4, NPC], mdt, kind="ExternalInput")
        lhsB_d = nc.dram_tensor("lhsB", [4, NPC], mdt, kind="ExternalInput")
        rhsH_d = nc.dram_tensor("rhsH", [4, M], mdt, kind="ExternalInput")
        rhsL_d = nc.dram_tensor("rhsL", [4, M], mdt, kind="ExternalInput")
    else:
        lhsT_d = nc.dram_tensor("lhsT", [KDIM, NPC], mdt, kind="ExternalInput")
        rhs_d = nc.dram_tensor("rhs", [KDIM, M], mdt, kind="ExternalInput")
    q2_d = nc.dram_tensor("q2rep", [128, RT * KNN], dt.float32,
                          kind="ExternalInput")
    out_d = nc.dram_tensor("out", [NPC, KNN], dt.float32, kind="ExternalOutput")

    with tile.TileContext(nc) as tc:
        with (
            tc.tile_pool(name="const", bufs=1) as constp,
            tc.tile_pool(name="psum", bufs=2, space="PSUM") as psump,
            tc.tile_pool(name="cand", bufs=3) as candp,
            tc.tile_pool(name="fin", bufs=1) as finp,
        ):
            q2_sb = constp.tile([128, RT * KNN], dt.float32)
            nc.sync.dma_start(out=q2_sb[:, :], in_=q2_d[:, :])
            # Replicate operands into each PE row-group's partition range.
            if mmdt == "f16x3":
                lhsA_sb = constp.tile([128, NPC], mdt)
                lhsB_sb = constp.tile([128, NPC], mdt)
                rhsH_sb = constp.tile([128, M], mdt)
                rhsL_sb = constp.tile([128, M], mdt)
                for i in range(PACK):
                    p = 32 * i
                    nc.sync.dma_start(out=lhsA_sb[p:p + 4, :], in_=lhsA_d[:, :])
                    nc.sync.dma_start(out=lhsB_sb[p:p + 4, :], in_=lhsB_d[:, :])
                    nc.sync.dma_start(out=rhsH_sb[p:p + 4, :], in_=rhsH_d[:, :])
                    nc.sync.dma_start(out=rhsL_sb[p:p + 4, :], in_=rhsL_d[:, :])
            else:
                rhs_sb = constp.tile([128, M], mdt)
                lhs_sb = constp.tile([128, NPC], mdt)
                for i in range(PACK):
                    nc.sync.dma_start(out=rhs_sb[32 * i:32 * i + KDIM, :], in_=rhs_d[:, :])
                    nc.sync.dma_start(out=lhs_sb[32 * i:32 * i + KDIM, :], in_=lhsT_d[:, :])

            s8_all = finp.tile([128, RT * KNN], dt.float32)
            for _rep in range(reps):
                for t in range(RT):
                    cands = candp.tile([128, NGROUP * KNN], dt.float32, tag="cands")
                    for g in range(NGROUP):
                        ps = psump.tile([128, GROUP], dt.float32, tag="ps")
                        for i in range(PACK):
                            c = g * PACK + i
                            p = 32 * i
                            tcol = slice(t * 128, (t + 1) * 128)
                            ccol = slice(c * CHUNK, (c + 1) * CHUNK)
                            pslice = ps[:, i * CHUNK:(i + 1) * CHUNK]
                            if mmdt == "f16x3":
                                for j, (lsb, rsb) in enumerate(
                                        [(lhsA_sb, rhsH_sb), (lhsB_sb, rhsH_sb),
                                         (lhsA_sb, rhsL_sb)]):
                                    nc.tensor.matmul(
                                        out=pslice,
                                        lhsT=lsb[p:p + 4, tcol],
                                        rhs=rsb[p:p + 4, ccol],
                                        start=(j == 0), stop=(j == 2),
                                        tile_position=(p, 0),
                                    )
                            else:
                                nc.tensor.matmul(
                                    out=pslice,
                                    lhsT=lhs_sb[p:p + 4, tcol],
                                    rhs=rhs_sb[p:p + 4, ccol],
                                    start=True, stop=True,
                                    tile_position=(p, 0),
                                )
                        nc.vector.max(out=cands[:, g * KNN:(g + 1) * KNN], in_=ps[:, :])
                    nc.vector.max(out=s8_all[:, t * KNN:(t + 1) * KNN], in_=cands[:, :])

            # Epilogue: d = sqrt(max(q2 - s, 0)); zero rows with min dist <= 0.25
            d2 = finp.tile([128, RT * KNN], dt.float32)
            nc.vector.tensor_sub(d2[:, :], q2_sb[:, :], s8_all[:, :])
            nc.vector.tensor_scalar_max(d2[:, :], d2[:, :], 0.0)
            dst = finp.tile([128, RT * KNN], dt.float32)
            nc.scalar.activation(dst[:, :], d2[:, :],
                                 mybir.ActivationFunctionType.Sqrt)
            good = finp.tile([128, RT], dt.float32)
            nc.vector.tensor_scalar(good[:, :], dst[:, 0:RT * KNN:KNN],
                                    OCC_RADIUS, None, mybir.AluOpType.is_gt)
            res = finp.tile([128, RT * KNN], dt.float32)
            nc.vector.tensor_tensor(
                res[:, :].rearrange("p (t j) -> p t j", j=KNN),
                dst[:, :].rearrange("p (t j) -> p t j", j=KNN),
                good[:, :, None].broadcast_to([128, RT, KNN]),
                mybir.AluOpType.mult,
            )
            nc.sync.dma_start(
                out=out_d.ap().rearrange("(t p) j -> p t j", p=128),
                in_=res[:, :].rearrange("p (t j) -> p t j", j=KNN),
            )

    nc.compile()
    _CACHE[key] = nc
    return nc


def _prep_in_maps(to_filter, target_coords, mmdt="f16x3"):
    q = np.ascontiguousarray(np.asarray(to_filter, dtype=np.float32)[:, :3])
    k = np.ascontiguousarray(np.asarray(target_coords, dtype=np.float32)[:, :3])
    q2 = np.sum(q * q, axis=1, dtype=np.float32)
    k2 = np.sum(k * k, axis=1, dtype=np.float32)
    in_maps = []
    if mmdt == "f16x3":
        qh = q.astype(np.float16)
        ql = (q - qh.astype(np.float32)).astype(np.float16)
        k2x = 2.0 * k.T
        kh = k2x.astype(np.float16)
        kl = (k2x - kh.astype(np.float32)).astype(np.float16)
        k2h = k2.astype(np.float16)
        k2l = (k2 - k2h.astype(np.float32)).astype(np.float16)
        lhsA = np.empty((4, N), np.float16)
        lhsA[0:3] = qh.T
        lhsA[3] = 1.0
        lhsB = np.empty((4, N), np.float16)
        lhsB[0:3] = ql.T
        lhsB[3] = 0.0
        rhsH = np.empty((4, M), np.float16)
        rhsH[0:3] = kh
        rhsH[3] = -k2h
        rhsL = np.empty((4, M), np.float16)
        rhsL[0:3] = kl
        rhsL[3] = -k2l
        for c in range(N_CORES):
            q2c = q2[c * NPC:(c + 1) * NPC].reshape(RT, 128).T
            in_maps.append({
                "lhsA": np.ascontiguousarray(lhsA[:, c * NPC:(c + 1) * NPC]),
                "lhsB": np.ascontiguousarray(lhsB[:, c * NPC:(c + 1) * NPC]),
                "rhsH": rhsH, "rhsL": rhsL,
                "q2rep": np.ascontiguousarray(np.repeat(q2c, KNN, axis=1)),
            })
        return in_maps
    lhsT_full = np.empty((4, N), np.float32)
    lhsT_full[0:3] = q.T
    lhsT_full[3] = 1.0
    rhs = np.empty((4, M), np.float32)
    rhs[0:3] = 2.0 * k.T
    rhs[3] = -k2
    for c in range(N_CORES):
        q2c = q2[c * NPC:(c + 1) * NPC].reshape(RT, 128).T  # [128, RT]
        q2rep = np.repeat(q2c, KNN, axis=1)                 # [128, RT*KNN]
        in_maps.append({
            "lhsT": np.ascontiguousarray(lhsT_full[:, c * NPC:(c + 1) * NPC]),
            "rhs": rhs,
            "q2rep": np.ascontiguousarray(q2rep),
        })
    return in_maps


def _run(to_filter, target_coords, trace=False, mmdt="f16x3"):
    from concourse import bass_utils

    nc = _build(mmdt=mmdt)
    in_maps = _prep_in_maps(to_filter, target_coords, mmdt=mmdt)
    res = bass_utils.run_bass_kernel_spmd(
        nc, in_maps, core_ids=list(range(N_CORES)), trace=trace,
    )
    out = np.concatenate([r["out"] for r in res.results], axis=0)
    return out, res


def kernel(to_filter, target_coords):
    out, _ = _run(to_filter, target_coords)
    return out



# revision 2
# speedup vs baseline: 1.0461x; 1.0461x over previous
"""Trainium2 Bass kernel for GuidedImplicitPointSampler KNN (top-8 + occupancy mask).

Strategy:
  - Shard the N=32768 query points across 8 NeuronCores (4096 each); every core
    holds the full M=16384 target cloud.
  - Per core, compute s[n,m] = 2*q_n.k_m - |k_m|^2 on the PE as a K=4 matmul
    (rows: 2kx,2ky,2kz,-|k|^2 moving; qx,qy,qz,1 stationary).  Since
    d2[n,m] = |q_n|^2 - s[n,m] and |q_n|^2 is constant per row, the 8 nearest
    targets are exactly the 8 LARGEST s values -> hardware top-8 (`nc.vector.max`)
    straight out of PSUM, no full distance matrix ever materialized.
  - K=4 uses only 4 of 128 PE rows, so 4 independent matmuls are packed into
    row-groups 0/32/64/96 via tile_position and run concurrently.
  - Epilogue on the tiny [128, 8] results: d = sqrt(max(q2 - s, 0)), zero rows
    whose nearest distance <= 0.25.
"""

import numpy as np

N = 32768
M = 16384
KNN = 8
OCC_RADIUS = 0.25
N_CORES = 8
NPC = N // N_CORES        # 4096 queries per core
RT = NPC // 128           # 32 row-tiles of 128 queries
CHUNK = 512               # matmul moving free dim (one PSUM bank)
PACK = 4                  # concurrent matmuls in PE row-groups
GROUP = CHUNK * PACK      # 2048 targets per PSUM tile
NGROUP = M // GROUP       # 8 groups per row-tile

_CACHE = {}


def _build(reps=1, mmdt="f16x3"):
    key = ("nc", reps, mmdt)
    if key in _CACHE:
        return _CACHE[key]
    from concourse import bacc, tile, mybir

    dt = mybir.dt
    # mmdt selects the PE path for s = 2q.k - |k|^2 (all run 1 cycle/row
    # except plain f32 which is 4):
    #   "f16k11": fp16 hi/lo split folded into ONE K=11 matmul:
    #            lhsT col = [qh(3), ql(3), qh(3), 1, 1],
    #            rhs col = [kh(3), kh(3), kl(3), -k2h, -k2l]  (error ~2^-22,
    #            same PE rate as K=4 since K<=32 fits one row-group).
    #   "f16x3": same terms as 3 separate accumulating matmuls (slower PE).
    #   "f32r":  fp32 bits at full PE rate, reduced internal precision (~2^-18).
    #   "f32":   exact fp32, 4x slower PE.
    mdt = {"f32r": dt.float32r, "f32": dt.float32, "f16x3": dt.float16,
           "f16k11": dt.float16}[mmdt]
    KDIM = 11 if mmdt == "f16k11" else 4
    nc = bacc.Bacc("TRN2", target_bir_lowering=False, debug=False,
                   num_devices=N_CORES)

    if mmdt == "f16x3":
        lhsA_d = nc.dram_tensor("lhsA", [4, NPC], mdt, kind="ExternalInput")
        lhsB_d = nc.dram_tensor("lhsB", [4, NPC], mdt, kind="ExternalInput")
        rhsH_d = nc.dram_tensor("rhsH", [4, M], mdt, kind="ExternalInput")
        rhsL_d = nc.dram_tensor("rhsL", [4, M], mdt, kind="ExternalInput")
    else:
        lhsT_d = nc.dram_tensor("lhsT", [KDIM, NPC], mdt, kind="ExternalInput")
        rhs_d = nc.dram_tensor("rhs", [KDIM, M], mdt, kind="ExternalInput")
    q2_d = nc.dram_tensor("q2rep", [128, RT * KNN], dt.float32,
                          kind="ExternalInput")
    out_d = nc.dram_tensor("out", [NPC, KNN], dt.float32, kind="ExternalOutput")

    with tile.TileContext(nc) as tc:
        with (
            tc.tile_pool(name="const", bufs=1) as constp,
            tc.tile_pool(name="psum", bufs=2, space="PSUM") as psump,
            tc.tile_pool(name="cand", bufs=3) as candp,
            tc.tile_pool(name="fin", bufs=1) as finp,
        ):
            q2_sb = constp.tile([128, RT * KNN], dt.float32)
            nc.sync.dma_start(out=q2_sb[:, :], in_=q2_d[:, :])
            # Replicate operands into each PE row-group's partition range.
            if mmdt == "f16x3":
                lhsA_sb = constp.tile([128, NPC], mdt)
                lhsB_sb = constp.tile([128, NPC], mdt)
                rhsH_sb = constp.tile([128, M], mdt)
                rhsL_sb = constp.tile([128, M], mdt)
                for i in range(PACK):
                    p = 32 * i
                    nc.sync.dma_start(out=lhsA_sb[p:p + 4, :], in_=lhsA_d[:, :])
                    nc.sync.dma_start(out=lhsB_sb[p:p + 4, :], in_=lhsB_d[:, :])
                    nc.sync.dma_start(out=rhsH_sb[p:p + 4, :], in_=rhsH_d[:, :])
                    nc.sync.dma_start(out=rhsL_sb[p:p + 4, :], in_=rhsL_d[:, :])
            else:
                rhs_sb = constp.tile([128, M], mdt)
                lhs_sb = constp.tile([128, NPC], mdt)
                for i in range(PACK):
                    nc.sync.dma_start(out=rhs_sb[32 * i:32 * i + KDIM, :], in_=rhs_d[:, :])
                    nc.sync.dma_start(out=lhs_sb[32 * i:32 * i + KDIM, :], in_=lhsT_d[:, :])

            s8_all = finp.tile([128, RT * KNN], dt.float32)
            for _rep in range(reps):
                for t in range(RT):
                    cands = candp.tile([128, NGROUP * KNN], dt.float32, tag="cands")
                    for g in range(NGROUP):
                        ps = psump.tile([128, GROUP], dt.float32, tag="ps")
                        for i in range(PACK):
                            c = g * PACK + i
                            p = 32 * i
                            tcol = slice(t * 128, (t + 1) * 128)
                            ccol = slice(c * CHUNK, (c + 1) * CHUNK)
                            pslice = ps[:, i * CHUNK:(i + 1) * CHUNK]
                            if mmdt == "f16x3":
                                for j, (lsb, rsb) in enumerate(
                                        [(lhsA_sb, rhsH_sb), (lhsB_sb, rhsH_sb),
                                         (lhsA_sb, rhsL_sb)]):
                                    nc.tensor.matmul(
                                        out=pslice,
                                        lhsT=lsb[p:p + 4, tcol],
                                        rhs=rsb[p:p + 4, ccol],
                                        start=(j == 0), stop=(j == 2),
                                        tile_position=(p, 0),
                                    )
                            else:
                                nc.tensor.matmul(
                                    out=pslice,
                                    lhsT=lhs_sb[p:p + 4, tcol],
                                    rhs=rhs_sb[p:p + 4, ccol],
                                    start=True, stop=True,
                                    tile_position=(p, 0),
                                )
                        nc.vector.max(out=cands[:, g * KNN:(g + 1) * KNN], in_=ps[:, :])
                    nc.vector.max(out=s8_all[:, t * KNN:(t + 1) * KNN], in_=cands[:, :])

            # Epilogue: d = sqrt(max(q2 - s, 0)); zero rows with min dist <= 0.25
            d2 = finp.tile([128, RT * KNN], dt.float32)
            nc.vector.tensor_sub(d2[:, :], q2_sb[:, :], s8_all[:, :])
            nc.vector.tensor_scalar_max(d2[:, :], d2[:, :], 0.0)
            dst = finp.tile([128, RT * KNN], dt.float32)
            nc.scalar.activation(dst[:, :], d2[:, :],
                                 mybir.ActivationFunctionType.Sqrt)
            good = finp.tile([128, RT], dt.float32)
            nc.vector.tensor_scalar(good[:, :], dst[:, 0:RT * KNN:KNN],
                                    OCC_RADIUS, None, mybir.AluOpType.is_gt)
            res = finp.tile([128, RT * KNN], dt.float32)
            nc.vector.tensor_tensor(
                res[:, :].rearrange("p (t j) -> p t j", j=KNN),
                dst[:, :].rearrange("p (t j) -> p t j", j=KNN),
                good[:, :, None].broadcast_to([128, RT, KNN]),
                mybir.AluOpType.mult,
            )
            nc.sync.dma_start(
                out=out_d.ap().rearrange("(t p) j -> p t j", p=128),
                in_=res[:, :].rearrange("p (t j) -> p t j", j=KNN),
            )

    nc.compile()
    _CACHE[key] = nc
    return nc


def _prep_in_maps(to_filter, target_coords, mmdt="f16x3"):
    q = np.ascontiguousarray(np.asarray(to_filter, dtype=np.float32)[:, :3])
    k = np.ascontiguousarray(np.asarray(target_coords, dtype=np.float32)[:, :3])
    q2 = np.sum(q * q, axis=1, dtype=np.float32)
    k2 = np.sum(k * k, axis=1, dtype=np.float32)
    in_maps = []
    if mmdt == "f16x3":
        qh = q.astype(np.float16)
        ql = (q - qh.astype(np.float32)).astype(np.float16)
        k2x = 2.0 * k.T
        kh = k2x.astype(np.float16)
        kl = (k2x - kh.astype(np.float32)).astype(np.float16)
        k2h = k2.astype(np.float16)
        k2l = (k2 - k2h.astype(np.float32)).astype(np.float16)
        lhsA = np.empty((4, N), np.float16)
        lhsA[0:3] = qh.T
        lhsA[3] = 1.0
        lhsB = np.empty((4, N), np.float16)
        lhsB[0:3] = ql.T
        lhsB[3] = 0.0
        rhsH = np.empty((4, M), np.float16)
        rhsH[0:3] = kh
        rhsH[3] = -k2h
        rhsL = np.empty((4, M), np.float16)
        rhsL[0:3] = kl
        rhsL[3] = -k2l
        for c in range(N_CORES):
            q2c = q2[c * NPC:(c + 1) * NPC].reshape(RT, 128).T
            in_maps.append({
                "lhsA": np.ascontiguousarray(lhsA[:, c * NPC:(c + 1) * NPC]),
                "lhsB": np.ascontiguousarray(lhsB[:, c * NPC:(c + 1) * NPC]),
                "rhsH": rhsH, "rhsL": rhsL,
                "q2rep": np.ascontiguousarray(np.repeat(q2c, KNN, axis=1)),
            })
        return in_maps
    if mmdt == "f16k11":
        # s = (qh+ql)·(2k)h + qh·(2k)l - k2h - k2l  (missing ql·(2k)l ~ 2^-22)
        qh = q.astype(np.float16)
        ql = (q - qh.astype(np.float32)).astype(np.float16)
        k2x = 2.0 * k.T
        kh = k2x.astype(np.float16)
        kl = (k2x - kh.astype(np.float32)).astype(np.float16)
        k2h = k2.astype(np.float16)
        k2l = (k2 - k2h.astype(np.float32)).astype(np.float16)
        lhsT = np.empty((11, N), np.float16)
        lhsT[0:3] = qh.T
        lhsT[3:6] = ql.T
        lhsT[6:9] = qh.T
        lhsT[9] = 1.0
        lhsT[10] = 1.0
        rhs = np.empty((11, M), np.float16)
        rhs[0:3] = kh
        rhs[3:6] = kh
        rhs[6:9] = kl
        rhs[9] = -k2h
        rhs[10] = -k2l
        for c in range(N_CORES):
            q2c = q2[c * NPC:(c + 1) * NPC].reshape(RT, 128).T
            in_maps.append({
                "lhsT": np.ascontiguousarray(lhsT[:, c * NPC:(c + 1) * NPC]),
                "rhs": rhs,
                "q2rep": np.ascontiguousarray(np.repeat(q2c, KNN, axis=1)),
            })
        return in_maps
    lhsT_full = np.empty((4, N), np.float32)
    lhsT_full[0:3] = q.T
    lhsT_full[3] = 1.0
    rhs = np.empty((4, M), np.float32)
    rhs[0:3] = 2.0 * k.T
    rhs[3] = -k2
    for c in range(N_CORES):
        q2c = q2[c * NPC:(c + 1) * NPC].reshape(RT, 128).T  # [128, RT]
        q2rep = np.repeat(q2c, KNN, axis=1)                 # [128, RT*KNN]
        in_maps.append({
            "lhsT": np.ascontiguousarray(lhsT_full[:, c * NPC:(c + 1) * NPC]),
            "rhs": rhs,
            "q2rep": np.ascontiguousarray(q2rep),
        })
    return in_maps


def _run(to_filter, target_coords, trace=False, mmdt="f16x3"):
    from concourse import bass_utils

    nc = _build(mmdt=mmdt)
    in_maps = _prep_in_maps(to_filter, target_coords, mmdt=mmdt)
    res = bass_utils.run_bass_kernel_spmd(
        nc, in_maps, core_ids=list(range(N_CORES)), trace=trace,
    )
    out = np.concatenate([r["out"] for r in res.results], axis=0)
    return out, res


def kernel(to_filter, target_coords):
    out, _ = _run(to_filter, target_coords)
    return out

